# revision 8
# baseline (speedup 1.0000x reference)
"""Trainium2 Bass kernel for a 3x3 stride-1 pad-1 Conv2d (NCHW).

Problem (hardcoded): x (16, 128, 128, 128) f32, K (3, 3, 128, 256) f32.
The reference reinterprets K's flat buffer as (Cin, kh, kw, Cout) and only
writes output rows/cols 0..124 (the rest of the 128x128 output stays zero).

Strategy: data-parallel over batch — 2 images per NeuronCore on 8 cores.
All device-side data is float16 (host pre-rounds; quantization ~2.5e-4 rel,
PSUM accumulation stays fp32). Per image the padded activation plane is
streamed in 16 chunks of 10 rows (8 output rows + 2 conv halo) so the first
matmul fires ~12us in instead of waiting ~30us for a whole-image DMA. The
conv is 9 accumulated matmuls (contraction over Cin=128) per output tile of
4 rows x 125 valid cols (N=500, one PSUM bank) per Cout half; cols >= 125
of the output are never computed (host zeroes them). fp16 weights get the
compiler's fast-weight-load path, hiding LDWEIGHTS under the matmul stream.
Outputs are staged per-chunk in SBUF (8 rows x 128 cols x fp16) and written
with one contiguous-2KB-per-partition DMA per (chunk, cout-half) on the
scalar-engine HWDGE ring, overlapping the input ring; the last chunk issues
per-row-block DMAs so the final transfer is tiny. A burst of dummy matmuls
warms the PE HAM clock gate (1.2 -> 2.4 GHz) while the first chunk DMA is
in flight.
"""

import numpy as np

import concourse.bacc as bacc
import concourse.mybir as mybir
import concourse.tile as tile
from concourse.bass_utils import run_bass_kernel_spmd

N_CORES = 8
B, CIN, H, W = 16, 128, 128, 128
COUT = 256
BPC = B // N_CORES  # images per core
HP, WP = H + 2, W + 2  # zero-padded plane
VALID = 125  # valid output rows/cols; rest is zero
NCHUNK = 16
CHUNK_ROWS = 8  # output rows per chunk
CHUNK_IN = CHUNK_ROWS + 2  # input rows per chunk (conv halo)
F32 = mybir.dt.float32
F16 = mybir.dt.float16

_NC_CACHE = {}


def _build_nc(reps=1):
    nc = bacc.Bacc()
    x_in = nc.dram_tensor("x", [BPC, CIN, HP, WP], F16, kind="ExternalInput")
    # Reference reinterprets K's flat buffer as (Cin, kh, kw, Cout); host
    # ships it as [Cin, (kh*3+kw)*256 + cout].
    w_in = nc.dram_tensor("w", [CIN, 9 * COUT], F16, kind="ExternalInput")
    out_t = nc.dram_tensor("out", [BPC, COUT, H, W], F16, kind="ExternalOutput")

    with tile.TileContext(nc) as tc:
        with (
            tc.tile_pool(name="wpool", bufs=1) as wpool,
            tc.tile_pool(name="xpool", bufs=8) as xpool,
            tc.tile_pool(name="opool", bufs=4) as opool,
            tc.tile_pool(name="pspool", bufs=8, space="PSUM") as pspool,
        ):
            # PE warmup: dummy matmuls with no DMA dependency keep the PE
            # busy through the HAM activity window until the first chunk
            # lands, so the real stream starts at 2.4 GHz.
            dummy = wpool.tile([CIN, 256], F16)
            nc.gpsimd.memset(dummy[:], 0.0)
            # Shares the "ps" tag (and thus slot set) with the conv tiles.
            wps = pspool.tile([128, 4, VALID], F32, name="ps")
            for _ in range(48):
                nc.tensor.matmul(
                    wps[:, 0:1, :],
                    dummy[:, 0:128],
                    dummy[:, 128 : 128 + VALID],
                    start=True,
                    stop=True,
                )

            # Weights on the scalar-engine HWDGE ring, chunk loads on the
            # sync-engine ring: the two transfers overlap.
            w_sb = wpool.tile([CIN, 9 * COUT], F16)
            nc.scalar.dma_start(out=w_sb[:], in_=w_in[:])

            for b in [b for _ in range(reps) for b in range(BPC)]:
                for c in range(NCHUNK):
                    xc = xpool.tile([CIN, CHUNK_IN, WP], F16)
                    nc.sync.dma_start(
                        out=xc[:],
                        in_=x_in[b, :, CHUNK_ROWS * c : CHUNK_ROWS * c + CHUNK_IN, :],
                    )
                    last = c == NCHUNK - 1
                    ocs = [
                        opool.tile([128, CHUNK_ROWS, W], F16, name=f"oc{c2}")
                        for c2 in range(2)
                    ]
                    if not last:
                        # Cols 125..127 are never computed but ride along in
                        # the contiguous store; zero them (host re-zeroes).
                        for oc in ocs:
                            nc.gpsimd.memset(oc[:, :, VALID:W], 0.0)
                    for rb in range(2):
                        r0 = 4 * rb  # chunk-local output row
                        # Global output rows 125..127 are never read; the
                        # final row-block computes only its single valid row.
                        nrows = 1 if last and rb == 1 else 4
                        for c2 in range(2):
                            ps = pspool.tile([128, nrows, VALID], F32)
                            for t in range(9):
                                kh, kw = divmod(t, 3)
                                c0 = t * COUT + c2 * 128
                                nc.tensor.matmul(
                                    ps[:],
                                    w_sb[:, c0 : c0 + 128],
                                    xc[:, r0 + kh : r0 + kh + nrows, kw : kw + VALID],
                                    start=(t == 0),
                                    stop=(t == 8),
                                )
                            nc.vector.tensor_copy(
                                out=ocs[c2][:, r0 : r0 + nrows, 0:VALID], in_=ps[:]
                            )
                            if last:
                                # Small per-cout-half stores on the (by now
                                # idle) sync ring keep the final DMA and its
                                # completion latency off the scalar ring's
                                # queue and short.
                                nc.sync.dma_start(
                                    out=out_t[
                                        b,
                                        c2 * 128 : (c2 + 1) * 128,
                                        CHUNK_ROWS * c + r0 : CHUNK_ROWS * c
                                        + r0
                                        + nrows,
                                        0:VALID,
                                    ],
                                    in_=ocs[c2][:, r0 : r0 + nrows, 0:VALID],
                                )
                    if not last:
                        for c2 in range(2):
                            nc.scalar.dma_start(
                                out=out_t[
                                    b,
                                    c2 * 128 : (c2 + 1) * 128,
                                    CHUNK_ROWS * c : CHUNK_ROWS * (c + 1),
                                    :,
                                ],
                                in_=ocs[c2][:],
                            )
    # Bacc defers register allocation and wait-splitting to compile(),
    # which finalize() runs; the SPMD exec path expects it done already.
    nc.finalize()
    return nc


def _get_nc(reps=1):
    if reps not in _NC_CACHE:
        _NC_CACHE[reps] = _build_nc(reps)
    return _NC_CACHE[reps]


def _run(x, K, trace=False, reps=1):
    x_pad = np.zeros((B, CIN, HP, WP), dtype=np.float16)
    x_pad[:, :, 1 : H + 1, 1 : W + 1] = np.asarray(x, dtype=np.float32).astype(
        np.float16
    )
    # Reference reinterprets K's flat buffer as (Cin, kh, kw, Cout); flat
    # (128, 2304) rows are Cin, cols are (kh*3+kw)*256 + cout.
    w_host = (
        np.asarray(K, dtype=np.float32).reshape(CIN, 9 * COUT).astype(np.float16)
    )
    in_maps = [
        {"x": x_pad[i * BPC : (i + 1) * BPC], "w": w_host} for i in range(N_CORES)
    ]
    res = run_bass_kernel_spmd(
        _get_nc(reps), in_maps, list(range(N_CORES)), trace=trace
    )
    out = np.concatenate(
        [res.results[i]["out"] for i in range(N_CORES)], axis=0
    ).astype(np.float32)
    # Device only writes valid rows/cols 0..124; zero the border strips.
    out[:, :, VALID:, :] = 0
    out[:, :, :, VALID:] = 0
    return out, res


def kernel(x, K):
    out, _ = _run(x, K, trace=False)
    return out


# revision 9
# speedup vs baseline: 1.0045x; 1.0045x over previous
"""Trainium2 Bass kernel for a 3x3 stride-1 pad-1 Conv2d (NCHW).

Problem (hardcoded): x (16, 128, 128, 128) f32, K (3, 3, 128, 256) f32.
The reference reinterprets K's flat buffer as (Cin, kh, kw, Cout) and only
writes output rows/cols 0..124 (the rest of the 128x128 output stays zero).

Strategy: data-parallel over batch — 2 images per NeuronCore on 8 cores.
All device-side data is float16 (host pre-rounds; quantization ~2.5e-4 rel,
PSUM accumulation stays fp32). Per image the padded activation plane is
streamed in 16 chunks of 10 rows (8 output rows + 2 conv halo); the first
chunk is further split into two 6-row mini-loads and the weights into two
cout-half DMAs so the first matmul's dependencies (first 6 input rows +
first cout-half weights) land ~11.5us in, instead of ~30us for whole-image
DMAs. The conv is 9 accumulated matmuls (contraction over Cin=128) per
output tile of 4 rows x 125 valid cols (N=500, one PSUM bank) per Cout
half; output cols >= 125 are never computed (host zeroes them). fp16
weights get the compiler's fast-weight-load path, hiding LDWEIGHTS under
the matmul stream. Outputs are staged per-chunk in SBUF (8 rows x 128 cols
x fp16) and written with one contiguous-2KB-per-partition DMA per (chunk,
cout-half) on the scalar-engine HWDGE ring, overlapping the input ring;
the last chunk issues small per-row-block stores on the (idle) sync ring
so the final transfer is tiny. A burst of dummy matmuls warms the PE HAM
clock gate (1.2 -> 2.4 GHz) while the first loads are in flight.
"""

import numpy as np

import concourse.bacc as bacc
import concourse.mybir as mybir
import concourse.tile as tile
from concourse.bass_utils import run_bass_kernel_spmd

N_CORES = 8
B, CIN, H, W = 16, 128, 128, 128
COUT = 256
BPC = B // N_CORES  # images per core
HP, WP = H + 2, W + 2  # zero-padded plane
VALID = 125  # valid output rows/cols; rest is zero
NCHUNK = 16
CHUNK_ROWS = 8  # output rows per chunk
CHUNK_IN = CHUNK_ROWS + 2  # input rows per chunk (conv halo)
F32 = mybir.dt.float32
F16 = mybir.dt.float16

_NC_CACHE = {}


def _build_nc(reps=1):
    nc = bacc.Bacc()
    x_in = nc.dram_tensor("x", [BPC, CIN, HP, WP], F16, kind="ExternalInput")
    # Weights laid out [Cin, (c2*9 + kh*3 + kw)*128 + cout%128] so each
    # cout-half is one contiguous DMA (see _run for the host reorder).
    w_in = nc.dram_tensor("w", [CIN, 9 * COUT], F16, kind="ExternalInput")
    out_t = nc.dram_tensor("out", [BPC, COUT, H, W], F16, kind="ExternalOutput")

    with tile.TileContext(nc) as tc:
        with (
            tc.tile_pool(name="wpool", bufs=1) as wpool,
            tc.tile_pool(name="xpool", bufs=8) as xpool,
            tc.tile_pool(name="opool", bufs=4) as opool,
            tc.tile_pool(name="pspool", bufs=8, space="PSUM") as pspool,
        ):
            # PE warmup: dummy matmuls with no DMA dependency keep the PE
            # busy through the HAM activity window until the first loads
            # land, so the real stream starts at 2.4 GHz.
            dummy = wpool.tile([CIN, 256], F16)
            nc.gpsimd.memset(dummy[:], 0.0)
            # Shares the "ps" tag (and thus slot set) with the conv tiles.
            wps = pspool.tile([128, 4, VALID], F32, name="ps")
            for _ in range(38):
                nc.tensor.matmul(
                    wps[:, 0:1, :],
                    dummy[:, 0:128],
                    dummy[:, 128 : 128 + VALID],
                    start=True,
                    stop=True,
                )

            # Weights on the scalar-engine HWDGE ring (cout-half 0 first so
            # the first matmul group is gated by a 0.3MB transfer, not the
            # full 0.6MB), chunk loads on the sync-engine ring: all overlap.
            w_sb = wpool.tile([CIN, 9 * COUT], F16)
            nc.scalar.dma_start(out=w_sb[:, 0 : 9 * 128], in_=w_in[:, 0 : 9 * 128])
            nc.scalar.dma_start(
                out=w_sb[:, 9 * 128 : 9 * 256], in_=w_in[:, 9 * 128 : 9 * 256]
            )

            first = True
            for b in [b for _ in range(reps) for b in range(BPC)]:
                for c in range(NCHUNK):
                    if first:
                        # Two 6-row mini-loads: the rb0 rows land (and gate
                        # the first matmul) in half the time of a 10-row
                        # chunk. parts[rb] = (tile, chunk-local start row).
                        xa = xpool.tile([CIN, 6, WP], F16, name="xc")
                        nc.sync.dma_start(out=xa[:], in_=x_in[b, :, 0:6, :])
                        xb = xpool.tile([CIN, 6, WP], F16, name="xc")
                        nc.sync.dma_start(out=xb[:], in_=x_in[b, :, 4:10, :])
                        parts = [(xa, 0), (xb, 4)]
                        first = False
                    else:
                        xc = xpool.tile([CIN, CHUNK_IN, WP], F16, name="xc")
                        nc.sync.dma_start(
                            out=xc[:],
                            in_=x_in[
                                b, :, CHUNK_ROWS * c : CHUNK_ROWS * c + CHUNK_IN, :
                            ],
                        )
                        parts = [(xc, 0), (xc, 0)]
                    last = c == NCHUNK - 1
                    ocs = [
                        opool.tile([128, CHUNK_ROWS, W], F16, name=f"oc{c2}")
                        for c2 in range(2)
                    ]
                    if not last:
                        # Cols 125..127 are never computed but ride along in
                        # the contiguous store; zero them (host re-zeroes).
                        for oc in ocs:
                            nc.gpsimd.memset(oc[:, :, VALID:W], 0.0)
                    for rb in range(2):
                        r0 = 4 * rb  # chunk-local output row
                        xt, xbase = parts[rb]
                        rl = r0 - xbase  # row offset within xt
                        # Global output rows 125..127 are never read; the
                        # final row-block computes only its single valid row.
                        nrows = 1 if last and rb == 1 else 4
                        for c2 in range(2):
                            ps = pspool.tile([128, nrows, VALID], F32)
                            for t in range(9):
                                kh, kw = divmod(t, 3)
                                c0 = (c2 * 9 + t) * 128
                                nc.tensor.matmul(
                                    ps[:],
                                    w_sb[:, c0 : c0 + 128],
                                    xt[:, rl + kh : rl + kh + nrows, kw : kw + VALID],
                                    start=(t == 0),
                                    stop=(t == 8),
                                )
                            nc.vector.tensor_copy(
                                out=ocs[c2][:, r0 : r0 + nrows, 0:VALID], in_=ps[:]
                            )
                            if last:
                                # Small per-row-block stores on the (by now
                                # idle) sync ring keep the final DMA and its
                                # completion latency short.
                                nc.sync.dma_start(
                                    out=out_t[
                                        b,
                                        c2 * 128 : (c2 + 1) * 128,
                                        CHUNK_ROWS * c + r0 : CHUNK_ROWS * c
                                        + r0
                                        + nrows,
                                        0:VALID,
                                    ],
                                    in_=ocs[c2][:, r0 : r0 + nrows, 0:VALID],
                                )
                    if not last:
                        for c2 in range(2):
                            nc.scalar.dma_start(
                                out=out_t[
                                    b,
                                    c2 * 128 : (c2 + 1) * 128,
                                    CHUNK_ROWS * c : CHUNK_ROWS * (c + 1),
                                    :,
                                ],
                                in_=ocs[c2][:],
                            )
    # Bacc defers register allocation and wait-splitting to compile(),
    # which finalize() runs; the SPMD exec path expects it done already.
    nc.finalize()
    return nc


def _get_nc(reps=1):
    if reps not in _NC_CACHE:
        _NC_CACHE[reps] = _build_nc(reps)
    return _NC_CACHE[reps]


def _run(x, K, trace=False, reps=1):
    x_pad = np.zeros((B, CIN, HP, WP), dtype=np.float16)
    x_pad[:, :, 1 : H + 1, 1 : W + 1] = np.asarray(x, dtype=np.float32).astype(
        np.float16
    )
    # Reference reinterprets K's flat buffer as (Cin, kh, kw, Cout): flat
    # (128, 9, 256) axes are (cin, tap, cout). Device wants cout-half-major
    # [cin, c2, tap, cout%128] so each half is one contiguous DMA.
    w9 = np.asarray(K, dtype=np.float32).reshape(CIN, 9, 2, 128)
    w_host = np.ascontiguousarray(w9.transpose(0, 2, 1, 3)).reshape(
        CIN, 9 * COUT
    ).astype(np.float16)
    in_maps = [
        {"x": x_pad[i * BPC : (i + 1) * BPC], "w": w_host} for i in range(N_CORES)
    ]
    res = run_bass_kernel_spmd(
        _get_nc(reps), in_maps, list(range(N_CORES)), trace=trace
    )
    out = np.concatenate(
        [res.results[i]["out"] for i in range(N_CORES)], axis=0
    ).astype(np.float32)
    # Device only writes valid rows/cols 0..124; zero the border strips.
    out[:, :, VALID:, :] = 0
    out[:, :, :, VALID:] = 0
    return out, res


def kernel(x, K):
    out, _ = _run(x, K, trace=False)
    return out


# revision 11
# speedup vs baseline: 1.0085x; 1.0039x over previous
"""Trainium2 Bass kernel for a 3x3 stride-1 pad-1 Conv2d (NCHW).

Problem (hardcoded): x (16, 128, 128, 128) f32, K (3, 3, 128, 256) f32.
The reference reinterprets K's flat buffer as (Cin, kh, kw, Cout) and only
writes output rows/cols 0..124 (the rest of the 128x128 output stays zero).

Strategy: data-parallel over batch — 2 images per NeuronCore on 8 cores.
All device-side data is float16 (host pre-rounds; quantization ~2.5e-4 rel,
PSUM accumulation stays fp32). Per image the padded activation plane is
streamed in 16 chunks of 10 rows (8 output rows + 2 conv halo); the first
chunk is further split into two 6-row mini-loads and the weights into two
cout-half DMAs so the first matmul's dependencies (first 6 input rows +
first cout-half weights) land ~11.5us in, instead of ~30us for whole-image
DMAs. The conv is 9 accumulated matmuls (contraction over Cin=128) per
output tile of 4 rows x 125 valid cols (N=500, one PSUM bank) per Cout
half; output cols >= 125 are never computed (host zeroes them). fp16
weights get the compiler's fast-weight-load path, hiding LDWEIGHTS under
the matmul stream. Outputs are staged per-chunk in SBUF (8 rows x 128 cols
x fp16) and written with one contiguous-2KB-per-partition DMA per (chunk,
cout-half) on the scalar-engine HWDGE ring, overlapping the input ring;
the last chunk issues small per-row-block stores on the (idle) sync ring
so the final transfer is tiny. A burst of dummy matmuls warms the PE HAM
clock gate (1.2 -> 2.4 GHz) while the first loads are in flight.
"""

import numpy as np

import concourse.bacc as bacc
import concourse.mybir as mybir
import concourse.tile as tile
from concourse.bass_utils import run_bass_kernel_spmd

N_CORES = 8
B, CIN, H, W = 16, 128, 128, 128
COUT = 256
BPC = B // N_CORES  # images per core
HP, WP = H + 2, W + 2  # zero-padded plane
VALID = 125  # valid output rows/cols; rest is zero
NCHUNK = 16
CHUNK_ROWS = 8  # output rows per chunk
CHUNK_IN = CHUNK_ROWS + 2  # input rows per chunk (conv halo)
F32 = mybir.dt.float32
F16 = mybir.dt.float16

_NC_CACHE = {}


def _build_nc(reps=1):
    nc = bacc.Bacc()
    x_in = nc.dram_tensor("x", [BPC, CIN, HP, WP], F16, kind="ExternalInput")
    # Weights laid out [Cin, (c2*9 + kh*3 + kw)*128 + cout%128] so each
    # cout-half is one contiguous DMA (see _run for the host reorder).
    w_in = nc.dram_tensor("w", [CIN, 9 * COUT], F16, kind="ExternalInput")
    out_t = nc.dram_tensor("out", [BPC, COUT, H, W], F16, kind="ExternalOutput")

    with tile.TileContext(nc) as tc:
        with (
            tc.tile_pool(name="wpool", bufs=1) as wpool,
            tc.tile_pool(name="xpool", bufs=8) as xpool,
            tc.tile_pool(name="opool", bufs=4) as opool,
            tc.tile_pool(name="pspool", bufs=8, space="PSUM") as pspool,
        ):
            # PE warmup: dummy matmuls with no DMA dependency keep the PE
            # busy through the HAM activity window until the first loads
            # land, so the real stream starts at 2.4 GHz.
            dummy = wpool.tile([CIN, 256], F16)
            nc.gpsimd.memset(dummy[:], 0.0)
            # Shares the "ps" tag (and thus slot set) with the conv tiles.
            wps = pspool.tile([128, 4, VALID], F32, name="ps")
            for _ in range(34):
                nc.tensor.matmul(
                    wps[:, 0:1, :],
                    dummy[:, 0:128],
                    dummy[:, 128 : 128 + VALID],
                    start=True,
                    stop=True,
                )

            # Weights on the scalar-engine HWDGE ring (cout-half 0 first so
            # the first matmul group is gated by a 0.3MB transfer, not the
            # full 0.6MB), chunk loads on the sync-engine ring: all overlap.
            w_sb = wpool.tile([CIN, 9 * COUT], F16)
            nc.scalar.dma_start(out=w_sb[:, 0 : 9 * 128], in_=w_in[:, 0 : 9 * 128])

            first = True
            for b in [b for _ in range(reps) for b in range(BPC)]:
                for c in range(NCHUNK):
                    if first:
                        # Two 6-row mini-loads: the rb0 rows land (and gate
                        # the first matmul) in half the time of a 10-row
                        # chunk. The second weight half rides the sync ring
                        # in parallel so the second cout-half group (needed
                        # ~2us into the stream) isn't gated by the scalar
                        # ring. parts[rb] = (tile, chunk-local start row).
                        xa = xpool.tile([CIN, 6, WP], F16, name="xc")
                        nc.sync.dma_start(out=xa[:], in_=x_in[b, :, 0:6, :])
                        nc.sync.dma_start(
                            out=w_sb[:, 9 * 128 : 9 * 256],
                            in_=w_in[:, 9 * 128 : 9 * 256],
                        )
                        xb = xpool.tile([CIN, 6, WP], F16, name="xc")
                        nc.sync.dma_start(out=xb[:], in_=x_in[b, :, 4:10, :])
                        parts = [(xa, 0), (xb, 4)]
                        first = False
                    else:
                        xc = xpool.tile([CIN, CHUNK_IN, WP], F16, name="xc")
                        nc.sync.dma_start(
                            out=xc[:],
                            in_=x_in[
                                b, :, CHUNK_ROWS * c : CHUNK_ROWS * c + CHUNK_IN, :
                            ],
                        )
                        parts = [(xc, 0), (xc, 0)]
                    last = c == NCHUNK - 1
                    ocs = [
                        opool.tile([128, CHUNK_ROWS, W], F16, name=f"oc{c2}")
                        for c2 in range(2)
                    ]
                    if not last:
                        # Cols 125..127 are never computed but ride along in
                        # the contiguous store; zero them (host re-zeroes).
                        for oc in ocs:
                            nc.gpsimd.memset(oc[:, :, VALID:W], 0.0)
                    for rb in range(2):
                        r0 = 4 * rb  # chunk-local output row
                        xt, xbase = parts[rb]
                        rl = r0 - xbase  # row offset within xt
                        # Global output rows 125..127 are never read; the
                        # final row-block computes only its single valid row.
                        nrows = 1 if last and rb == 1 else 4
                        for c2 in range(2):
                            ps = pspool.tile([128, nrows, VALID], F32)
                            for t in range(9):
                                kh, kw = divmod(t, 3)
                                c0 = (c2 * 9 + t) * 128
                                nc.tensor.matmul(
                                    ps[:],
                                    w_sb[:, c0 : c0 + 128],
                                    xt[:, rl + kh : rl + kh + nrows, kw : kw + VALID],
                                    start=(t == 0),
                                    stop=(t == 8),
                                )
                            nc.vector.tensor_copy(
                                out=ocs[c2][:, r0 : r0 + nrows, 0:VALID], in_=ps[:]
                            )
                            if last:
                                # Small per-row-block stores on the (by now
                                # idle) sync ring keep the final DMA and its
                                # completion latency short.
                                nc.sync.dma_start(
                                    out=out_t[
                                        b,
                                        c2 * 128 : (c2 + 1) * 128,
                                        CHUNK_ROWS * c + r0 : CHUNK_ROWS * c
                                        + r0
                                        + nrows,
                                        0:VALID,
                                    ],
                                    in_=ocs[c2][:, r0 : r0 + nrows, 0:VALID],
                                )
                    if not last:
                        for c2 in range(2):
                            nc.scalar.dma_start(
                                out=out_t[
                                    b,
                                    c2 * 128 : (c2 + 1) * 128,
                                    CHUNK_ROWS * c : CHUNK_ROWS * (c + 1),
                                    :,
                                ],
                                in_=ocs[c2][:],
                            )
    # Bacc defers register allocation and wait-splitting to compile(),
    # which finalize() runs; the SPMD exec path expects it done already.
    nc.finalize()
    return nc


def _get_nc(reps=1):
    if reps not in _NC_CACHE:
        _NC_CACHE[reps] = _build_nc(reps)
    return _NC_CACHE[reps]


def _run(x, K, trace=False, reps=1):
    x_pad = np.zeros((B, CIN, HP, WP), dtype=np.float16)
    x_pad[:, :, 1 : H + 1, 1 : W + 1] = np.asarray(x, dtype=np.float32).astype(
        np.float16
    )
    # Reference reinterprets K's flat buffer as (Cin, kh, kw, Cout): flat
    # (128, 9, 256) axes are (cin, tap, cout). Device wants cout-half-major
    # [cin, c2, tap, cout%128] so each half is one contiguous DMA.
    w9 = np.asarray(K, dtype=np.float32).reshape(CIN, 9, 2, 128)
    w_host = np.ascontiguousarray(w9.transpose(0, 2, 1, 3)).reshape(
        CIN, 9 * COUT
    ).astype(np.float16)
    in_maps = [
        {"x": x_pad[i * BPC : (i + 1) * BPC], "w": w_host} for i in range(N_CORES)
    ]
    res = run_bass_kernel_spmd(
        _get_nc(reps), in_maps, list(range(N_CORES)), trace=trace
    )
    out = np.concatenate(
        [res.results[i]["out"] for i in range(N_CORES)], axis=0
    ).astype(np.float32)
    # Device only writes valid rows/cols 0..124; zero the border strips.
    out[:, :, VALID:, :] = 0
    out[:, :, :, VALID:] = 0
    return out, res


def kernel(x, K):
    out, _ = _run(x, K, trace=False)
    return out
